# revision 5
# baseline (speedup 1.0000x reference)
"""Trainium2 Bass kernel for nn_CombinedAMLModel (dense_mlp, 8 NeuronCores).

Sharding: tensor-parallel over the gene axis (20000 genes -> 2500 per core).

Per core:
  Phase A  - per-(tech,gene) 1->4->1 MLPs plus the per-gene tech combinor,
             computed as 12 relu-affine passes (genes on partitions, per-
             partition scale/bias on ACT/DVE), accumulated into PSUM with
             diagonal fp32r matmuls whose diagonals carry W2[t,g,h]*Wc[g,t].
             The constant term (sum_t b2*Wc + bc) is added during the
             PSUM->SBUF copy. Produces z[g_local, s] (2500 x 1024).
  Phase B  - out1T[n, s] += CW0T[g, n].T @ z[g, s]  (fp32r, K=2500 local
             genes, n=2000), written to DRAM as this core's partial.
  Phase C  - AllReduce of the (2000, 1024) partials across 8 cores.
  Phase D  - tail MLP 2000->200->20->1, replicated on every core, computed
             entirely in transposed orientation (layer outputs on partitions,
             samples on the free axis) so no transposes are needed anywhere.

All matmuls run in float32r (full-rate fp32 matmul, ~1e-4 relative error).
"""
import os
import sys

sys.path.insert(0, "/opt/trn_rl_repo")

import numpy as np
from contextlib import ExitStack

import concourse.bass as bass
import concourse.tile as tile
from concourse import bacc, mybir
from concourse.bass_utils import run_bass_kernel_spmd

T, S, G, H = 3, 1024, 20000, 4
NCORES = 8
GL = G // NCORES            # genes per core
PT = 125                    # gene-tile partition size
NGT = GL // PT              # gene tiles per core
NK = T * H                  # local relu-affine passes
N1, N2, N3 = 2000, 200, 20
PN = 125                    # n-tile partition size for layer-1 output
NNT = N1 // PN              # n tiles
SH = 512                    # PSUM-bank half of the sample axis
ACT_KS = frozenset((0, 2, 4, 6, 8, 10))   # passes on ScalarE; rest on VectorE

f32 = mybir.dt.float32
f32r = mybir.dt.float32r

LAST_RUN = {}
_CACHE = {}


def _build_program():
    nc = bacc.Bacc("TRN2", target_bir_lowering=False, debug=False,
                   num_devices=NCORES)
    d = {}

    def inp(name, shape, dt=f32):
        d[name] = nc.dram_tensor(name, list(shape), dt, kind="ExternalInput").ap()

    inp("xT", (T, GL, S))
    inp("scl", (PT, NGT * NK))
    inp("bia", (PT, NGT * NK))
    inp("cst", (PT, NGT))
    inp("diag", (PT, NGT * NK * PT), f32r)
    inp("cw0t", (GL, N1), f32r)
    inp("cb0", (PN, NNT))
    inp("cw1t", (N1, N2), f32r)
    inp("cb1", (100, 2))
    inp("cw2t", (N2, N3), f32r)
    inp("cb2", (N3, 1))
    inp("cwft", (N3, 1), f32r)
    inp("cbf", (1, 1))
    out_d = nc.dram_tensor("out", [1, S], f32, kind="ExternalOutput").ap()

    Relu = mybir.ActivationFunctionType.Relu
    Ident = mybir.ActivationFunctionType.Identity

    with tile.TileContext(nc) as tc, ExitStack() as ctx:
        const = ctx.enter_context(tc.tile_pool(name="const", bufs=1))
        xpool = ctx.enter_context(tc.tile_pool(name="x", bufs=6))
        dpool = ctx.enter_context(tc.tile_pool(name="diag", bufs=2))
        apool = ctx.enter_context(tc.tile_pool(name="a", bufs=3))
        vpool = ctx.enter_context(tc.tile_pool(name="v", bufs=2))
        zpool = ctx.enter_context(tc.tile_pool(name="z", bufs=NGT))
        wpool = ctx.enter_context(tc.tile_pool(name="w0", bufs=2))
        opool = ctx.enter_context(tc.tile_pool(name="o1", bufs=3))
        w1pool = ctx.enter_context(tc.tile_pool(name="w1", bufs=NNT))
        tpool = ctx.enter_context(tc.tile_pool(name="tail", bufs=1))
        zps = ctx.enter_context(tc.tile_pool(name="zps", bufs=4, space="PSUM"))
        mmps = ctx.enter_context(tc.tile_pool(name="mmps", bufs=4, space="PSUM"))
        dram = ctx.enter_context(tc.tile_pool(name="dram", bufs=1, space="DRAM"))

        sclt = const.tile([PT, NGT * NK], f32)
        nc.sync.dma_start(sclt[:], d["scl"][:])
        biat = const.tile([PT, NGT * NK], f32)
        nc.sync.dma_start(biat[:], d["bia"][:])
        cstt = const.tile([PT, NGT], f32)
        nc.sync.dma_start(cstt[:], d["cst"][:])
        cb0t = const.tile([PN, NNT], f32)
        nc.sync.dma_start(cb0t[:], d["cb0"][:])
        cb1t = const.tile([100, 2], f32)
        nc.sync.dma_start(cb1t[:], d["cb1"][:])
        cb2t = const.tile([N3, 1], f32)
        nc.sync.dma_start(cb2t[:], d["cb2"][:])
        cwftt = const.tile([N3, 1], f32r)
        nc.sync.dma_start(cwftt[:], d["cwft"][:])
        cbft = const.tile([1, 1], f32)
        nc.sync.dma_start(cbft[:], d["cbf"][:])
        cw2tt = const.tile([100, 2 * N3], f32r)
        for mc in range(2):
            nc.sync.dma_start(cw2tt[:, mc * N3:(mc + 1) * N3],
                              d["cw2t"][mc * 100:(mc + 1) * 100, :])

        partial = dram.tile([N1, S], f32)
        summed = dram.tile([N1, S], f32)

        # ---------------- Phase A: local gene MLPs + combinor ----------------
        cw0_r = d["cw0t"].rearrange("(gt p) n -> p gt n", p=PT)
        z_tiles = []
        for gt in range(NGT):
            xts = []
            for t in range(T):
                xt = xpool.tile([PT, S], f32, tag="x")
                nc.sync.dma_start(xt[:], d["xT"][t, gt * PT:(gt + 1) * PT, :])
                xts.append(xt)
            dg = dpool.tile([PT, NK * PT], f32r, tag="diag")
            nc.sync.dma_start(dg[:],
                              d["diag"][:, gt * NK * PT:(gt + 1) * NK * PT])
            pss = (zps.tile([PT, SH], f32, tag="zps", name=f"zps{gt}_0"),
                   zps.tile([PT, SH], f32, tag="zps", name=f"zps{gt}_1"))
            for k in range(NK):
                t = k // H
                ci = gt * NK + k
                if k in ACT_KS:
                    a = apool.tile([PT, S], f32r, tag="a")
                    nc.scalar.activation(a[:], xts[t][:], Relu,
                                         bias=biat[:, ci:ci + 1],
                                         scale=sclt[:, ci:ci + 1])
                else:
                    v = vpool.tile([PT, S], f32, tag="v")
                    nc.vector.tensor_scalar(v[:], xts[t][:],
                                            sclt[:, ci:ci + 1],
                                            biat[:, ci:ci + 1],
                                            mybir.AluOpType.mult,
                                            mybir.AluOpType.add)
                    a = apool.tile([PT, S], f32r, tag="a")
                    nc.vector.tensor_scalar(a[:], v[:], 0.0, None,
                                            mybir.AluOpType.max)
                for sh in range(2):
                    nc.tensor.matmul(pss[sh][:],
                                     dg[:, k * PT:(k + 1) * PT],
                                     a[:, sh * SH:(sh + 1) * SH],
                                     start=(k == 0), stop=(k == NK - 1))
            z = zpool.tile([PT, S], f32r, tag="z")
            for sh in range(2):
                nc.scalar.activation(z[:, sh * SH:(sh + 1) * SH], pss[sh][:],
                                     Ident, bias=cstt[:, gt:gt + 1], scale=1.0)
            z_tiles.append(z)

        # ---------------- Phase B: out1T = CW0 @ z (local-gene partial) ------
        for nt in range(NNT):
            w = wpool.tile([PT, NGT, PN], f32r, tag="w0")
            nc.sync.dma_start(w[:], cw0_r[:, :, nt * PN:(nt + 1) * PN])
            pp = (mmps.tile([PN, SH], f32, tag="mm", name=f"mm{nt}_0"),
                  mmps.tile([PN, SH], f32, tag="mm", name=f"mm{nt}_1"))
            for gt in range(NGT):
                for sh in range(2):
                    nc.tensor.matmul(pp[sh][:], w[:, gt, :],
                                     z_tiles[gt][:, sh * SH:(sh + 1) * SH],
                                     start=(gt == 0), stop=(gt == NGT - 1))
            o = opool.tile([PN, S], f32, tag="o1")
            for sh in range(2):
                nc.scalar.copy(o[:, sh * SH:(sh + 1) * SH], pp[sh][:])
            nc.sync.dma_start(partial[nt * PN:(nt + 1) * PN, :], o[:])

        # ---------------- Phase C: AllReduce across the 8 cores --------------
        nc.gpsimd.collective_compute(
            "AllReduce", mybir.AluOpType.add,
            replica_groups=[list(range(NCORES))],
            ins=[partial.opt()], outs=[summed.opt()],
        )

        # ---------------- Phase D: replicated tail 2000->200->20->1 ----------
        w1_tiles = []
        for kt in range(NNT):
            w1 = w1pool.tile([PN, N2], f32r, tag="w1")
            nc.sync.dma_start(w1[:], d["cw1t"][kt * PN:(kt + 1) * PN, :])
            w1_tiles.append(w1)
        z1_tiles = []
        for kt in range(NNT):
            y1 = xpool.tile([PN, S], f32, tag="x")
            nc.sync.dma_start(y1[:], summed[kt * PN:(kt + 1) * PN, :])
            z1 = zpool.tile([PN, S], f32r, tag="z")
            nc.scalar.activation(z1[:], y1[:], Relu,
                                 bias=cb0t[:, kt:kt + 1], scale=1.0)
            z1_tiles.append(z1)
        z2all = tpool.tile([100, 2 * S], f32r, tag="z2")
        z2_tiles = [z2all[:, 0:S], z2all[:, S:2 * S]]
        for mc in range(2):
            z2 = z2_tiles[mc]
            for sh in range(2):
                ps = mmps.tile([100, SH], f32, tag="mm")
                for kt in range(NNT):
                    nc.tensor.matmul(ps[:],
                                     w1_tiles[kt][:, mc * 100:(mc + 1) * 100],
                                     z1_tiles[kt][:, sh * SH:(sh + 1) * SH],
                                     start=(kt == 0), stop=(kt == NNT - 1))
                nc.scalar.activation(z2[:, sh * SH:(sh + 1) * SH], ps[:], Relu,
                                     bias=cb1t[:, mc:mc + 1], scale=1.0)
        z3 = tpool.tile([N3, S], f32r, tag="z3")
        for sh in range(2):
            ps = mmps.tile([N3, SH], f32, tag="mm")
            for mc in range(2):
                nc.tensor.matmul(ps[:], cw2tt[:, mc * N3:(mc + 1) * N3],
                                 z2_tiles[mc][:, sh * SH:(sh + 1) * SH],
                                 start=(mc == 0), stop=(mc == 1))
            nc.scalar.activation(z3[:, sh * SH:(sh + 1) * SH], ps[:], Relu,
                                 bias=cb2t[:], scale=1.0)
        outt = tpool.tile([1, S], f32, tag="outt")
        for sh in range(2):
            ps = mmps.tile([1, SH], f32, tag="mm")
            nc.tensor.matmul(ps[:], cwftt[:],
                             z3[:, sh * SH:(sh + 1) * SH],
                             start=True, stop=True)
            nc.scalar.activation(outt[:, sh * SH:(sh + 1) * SH], ps[:], Ident,
                                 bias=cbft[:], scale=1.0)
        nc.sync.dma_start(out_d[:], outt[:])

    nc.compile()
    return nc


def _shard_inputs(x, W1, b1, W2, b2, Wc, bc,
                  CW0, Cb0, CW1, Cb1, CW2, Cb2, CWf, Cbf):
    f = lambda a: np.ascontiguousarray(a, dtype=np.float32)
    shared = {
        "cb0": f(Cb0.reshape(NNT, PN).T),
        "cw1t": f(CW1.T),
        "cb1": f(Cb1.reshape(2, 100).T),
        "cw2t": f(CW2.T),
        "cb2": f(Cb2.reshape(N3, 1)),
        "cwft": f(CWf.T),
        "cbf": f(Cbf.reshape(1, 1)),
    }
    in_maps = []
    for c in range(NCORES):
        gs = slice(c * GL, (c + 1) * GL)
        scl = W1[:, gs, :].transpose(1, 0, 2).reshape(GL, NK)
        bia = b1[:, gs, :].transpose(1, 0, 2).reshape(GL, NK)
        coe = (W2[:, gs, :] * Wc[gs, :].T[:, :, None]) \
            .transpose(1, 0, 2).reshape(GL, NK)
        cst = (b2[:, gs] * Wc[gs, :].T).sum(0) + bc[gs]
        coep = coe.reshape(NGT, PT, NK)
        diag = np.zeros((PT, NGT, NK, PT), np.float32)
        for p in range(PT):
            diag[p, :, :, p] = coep[:, p, :]
        in_maps.append({
            "xT": f(x[:, :, gs].transpose(0, 2, 1)),
            "scl": f(scl.reshape(NGT, PT, NK).transpose(1, 0, 2)
                     .reshape(PT, NGT * NK)),
            "bia": f(bia.reshape(NGT, PT, NK).transpose(1, 0, 2)
                     .reshape(PT, NGT * NK)),
            "cst": f(cst.reshape(NGT, PT).T),
            "diag": diag.reshape(PT, NGT * NK * PT),
            "cw0t": f(CW0[:, gs].T),
            **shared,
        })
    return in_maps


def _install_profile_shim():
    """Register the NTFF profiling hook that this container's antenv lacks.

    bass_utils' trace path imports antenv.axon_hooks; the boot helper that
    can construct the actual hook exists, so wire it up dynamically.
    """
    import types
    try:
        import antenv.axon_hooks  # noqa: F401
        return True
    except ImportError:
        pass
    try:
        import antenv
        from trn_agent_boot.trn_boot import _ntff_profile_via_ctypes
        hook = _ntff_profile_via_ctypes("/opt/axon/libaxon_pjrt.so")
        mod = types.ModuleType("antenv.axon_hooks")
        mod.get_axon_ntff_profile_hook = lambda: hook
        mod.set_axon_ntff_profile_hook = lambda h: None
        sys.modules["antenv.axon_hooks"] = mod
        antenv.axon_hooks = mod
        return hook is not None
    except Exception:
        return False


def kernel(**inputs):
    inputs = {k: np.asarray(v) for k, v in inputs.items()}
    in_maps = _shard_inputs(**inputs)
    if "nc" not in _CACHE:
        _CACHE["nc"] = _build_program()
    nc = _CACHE["nc"]
    trace = bool(os.environ.get("KERNEL_PROFILE")) and _install_profile_shim()
    res = run_bass_kernel_spmd(nc, in_maps, core_ids=list(range(NCORES)),
                               trace=trace)
    LAST_RUN["exec_time_ns"] = res.exec_time_ns
    LAST_RUN["mean_exec_time_ns"] = res.mean_exec_time_ns
    if res.instructions_and_trace is not None:
        LAST_RUN["trace_path"] = res.instructions_and_trace[1]
    return res.results[0]["out"].reshape(1, S, 1)


if __name__ == "__main__":
    rng = np.random.default_rng(0)
    ins = {
        "x": rng.standard_normal((T, S, G), dtype=np.float32),
        "W1": rng.standard_normal((T, G, H), dtype=np.float32) * 0.5,
        "b1": rng.standard_normal((T, G, H), dtype=np.float32) * 0.1,
        "W2": rng.standard_normal((T, G, H), dtype=np.float32) * 0.5,
        "b2": rng.standard_normal((T, G), dtype=np.float32) * 0.1,
        "Wc": rng.standard_normal((G, T), dtype=np.float32) * 0.5,
        "bc": rng.standard_normal((G,), dtype=np.float32) * 0.1,
        "CW0": rng.standard_normal((N1, G), dtype=np.float32) * 0.007,
        "Cb0": rng.standard_normal((N1,), dtype=np.float32) * 0.007,
        "CW1": rng.standard_normal((N2, N1), dtype=np.float32) * 0.02,
        "Cb1": rng.standard_normal((N2,), dtype=np.float32) * 0.02,
        "CW2": rng.standard_normal((N3, N2), dtype=np.float32) * 0.07,
        "Cb2": rng.standard_normal((N3,), dtype=np.float32) * 0.07,
        "CWf": rng.standard_normal((1, N3), dtype=np.float32) * 0.2,
        "Cbf": rng.standard_normal((1,), dtype=np.float32) * 0.2,
    }
    out = kernel(**ins)
    # numpy reference
    xx = ins["x"]
    h = np.maximum(xx[..., None] * ins["W1"][:, None] + ins["b1"][:, None], 0.0)
    y = np.einsum("tsgh,tgh->tsg", h, ins["W2"]) + ins["b2"][:, None, :]
    zz = np.einsum("tsg,gt->sg", y, ins["Wc"]) + ins["bc"]
    for Wl, bl in ((ins["CW0"], ins["Cb0"]), (ins["CW1"], ins["Cb1"]),
                   (ins["CW2"], ins["Cb2"])):
        zz = np.maximum(zz @ Wl.T + bl, 0.0)
    ref = (zz @ ins["CWf"].T + ins["Cbf"])[None]
    err = np.abs(out - ref).max() / (np.abs(ref).max() + 1e-12)
    print("self-test rel err:", err)
    print("exec_time_ns:", LAST_RUN.get("exec_time_ns"))
